# revision 47
# baseline (speedup 1.0000x reference)
"""Multi-head attention (B=2, S=2048, EMB=1024, 16 heads) on 8 Trainium2 cores.

Sharding: core c -> batch c//4, head-group c%4 (4 heads = 256 projection dims).
Each core computes Q/K/V projections for its head group in transposed layout
(Q^T, K^T with head-dim on partitions; V natural), attention without max
subtraction (scores ~ N(0,1), exp never overflows in fp32), the softmax
denominator via a ones-column appended to V (free inside the ctx matmul), and
a row-parallel partial of the output projection.  The host sums the 4 partials
per batch and adds the output bias (the all-reduce of the row-parallel fc_out
is done at unshard time; no device collectives needed).

All matmuls run as float32r (full-rate on the PE at N>=256).  The BIR verifier
requires fp32r matmul operands to be *produced* as fp32r, so every
matmul-feeding tensor is declared fp32r end-to-end (same 4-byte layout as
fp32; host passes float32 arrays).
"""

import numpy as np

import concourse.tile as tile
from concourse import bacc, mybir
from concourse import bass_utils

EMB = 1024
S = 2048
B = 2
HPC = 4            # heads per core
DQ = HPC * 64      # 256 projection dims per core
NCORES = 8

F32 = mybir.dt.float32
F32R = mybir.dt.float32r
EXP = mybir.ActivationFunctionType.Exp

KT_E = EMB // 128  # 8 contraction tiles over EMB
NQC = S // 512     # 4 query chunks
NST = S // 128     # 16 sequence tiles

_NC = None
TRACE = False
LAST_RESULT = None
_ABLATE = None  # None = all phases; else subset of {"kv", "q", "attn", "out"}
_CTX_PROBE = None
_SPLIT = set()  # heterogeneous in-group splits are rejected by HW; keep off


def _on(phase):
    return _ABLATE is None or phase in _ABLATE


def _mha(ctx, tc, xqT, xkT, xvT, wqT, wkT, wvT, woT, bq, bk, bv, out, bench_iters=None):
    nc = tc.nc

    cpool = ctx.enter_context(tc.tile_pool(name="const", bufs=1))
    xpool = ctx.enter_context(tc.tile_pool(name="xin", bufs=16))
    epool = ctx.enter_context(tc.tile_pool(name="exp", bufs=6))
    bpool = ctx.enter_context(tc.tile_pool(name="bcsb", bufs=2))
    opool = ctx.enter_context(tc.tile_pool(name="osb", bufs=3))
    sc_ps = ctx.enter_context(tc.tile_pool(name="scps", bufs=2, space="PSUM"))
    ctx_ps = ctx.enter_context(tc.tile_pool(name="ctxps", bufs=4, space="PSUM"))
    mm_ps = sc_ps
    bc_ps = sc_ps

    # ---- persistent SBUF tensors ----
    ones_row = cpool.tile([1, 512], F32R)
    nc.vector.memset(ones_row[:].bitcast(F32), 1.0)
    sel64 = cpool.tile([65, 64], F32R)          # one-hot: row 64 -> all cols
    nc.vector.memset(sel64[:].bitcast(F32), 0.0)
    nc.vector.memset(sel64[64:65, :].bitcast(F32), 1.0)
    rden = cpool.tile([65, 512], F32R)          # row 64 = denom, rows 0..63 zero
    nc.vector.memset(rden[0:64, :].bitcast(F32), 0.0)

    wq_sb = cpool.tile([128, KT_E * DQ], F32R)  # [128, 2048]: wq_sb[p, n*256+m] = WqT[n*128+p, m]
    wk_sb = cpool.tile([128, KT_E * DQ], F32R)
    wv_sb = cpool.tile([128, KT_E * DQ], F32R)
    for sb, src in ((wq_sb, wqT), (wk_sb, wkT), (wv_sb, wvT)):
        nc.sync.dma_start(
            sb[:].rearrange("p (n m) -> p n m", n=KT_E),
            src.rearrange("(n p) m -> p n m", p=128),
        )
    wo_sb = cpool.tile([128, 2 * EMB], F32R)    # wo_sb[p, n*1024+f] = WoT[n*128+p, f]
    nc.sync.dma_start(
        wo_sb[:].rearrange("p (n m) -> p n m", n=2),
        woT.rearrange("(n p) m -> p n m", p=128),
    )
    bq_sb = cpool.tile([1, DQ], F32R)
    bk_sb = cpool.tile([1, DQ], F32R)
    bv_sb = cpool.tile([1, DQ], F32R)
    for sb, src in ((bq_sb, bq), (bk_sb, bk), (bv_sb, bv)):
        nc.sync.dma_start(sb[:], src[:])

    # results of phase 1/2 kept resident
    kT_sb = cpool.tile([128, 2 * S], F32R)      # [dq-block 2][s 2048]
    qT_sb = cpool.tile([128, 2 * S], F32R)
    ctxT_sb = cpool.tile([128, 2 * S], F32R)
    v_sb = cpool.tile([128, NST * (HPC * 65)], F32R)  # per s-tile: 4 heads x (64 V + ones col)
    nc.vector.memset(
        v_sb[:].bitcast(F32).rearrange("p (t h m) -> p t h m", t=NST, h=HPC)[:, :, :, 64:65],
        1.0,
    )

    if _ABLATE:
        # ablation: un-computed persistent tensors need defined contents
        for t in (kT_sb, qT_sb, ctxT_sb, v_sb):
            nc.vector.memset(t[:].bitcast(F32), 0.001)
        nc.vector.memset(rden[64:65, :].bitcast(F32), 1.0)

    def body():
        _body(tc, nc, xqT, xkT, xvT, out, ones_row, sel64, rden, wq_sb, wk_sb,
              wv_sb, wo_sb, bq_sb, bk_sb, bv_sb, kT_sb, qT_sb, ctxT_sb, v_sb,
              xpool, epool, bpool, opool, mm_ps, sc_ps, ctx_ps, bc_ps)

    if bench_iters:
        hints = (
            mybir.EngineType.PE,
            mybir.EngineType.Activation,
            mybir.EngineType.DVE,
            mybir.EngineType.SP,
            mybir.EngineType.Pool,
        )
        with tc.For_i(0, bench_iters, 1, hint_engines=hints):
            body()
    else:
        body()


def _proj_chain(nc, ps, w_sb, xs, dq, b_sb, ones_row):
    """Q/K projection chain into psum ps: kt=0 full K=128 (group start, marks
    every element written), then K=64 halves on alternating row groups (they
    overlap in the PE array), bias last."""
    col = lambda kt: kt * DQ + dq * 128
    nc.tensor.matmul(ps[:], w_sb[:, col(0): col(0) + 128], xs[0][:],
                     start=True, stop=False)
    for kt in range(1, KT_E):
        if "proj" in _SPLIT:
            for b in (0, 64):
                nc.tensor.matmul(
                    ps[:], w_sb[b:b + 64, col(kt): col(kt) + 128], xs[kt][b:b + 64, :],
                    start=False, stop=False,
                )
        else:
            nc.tensor.matmul(
                ps[:], w_sb[:, col(kt): col(kt) + 128], xs[kt][:],
                start=False, stop=False,
            )
    nc.tensor.matmul(
        ps[:], b_sb[0:1, dq * 128: dq * 128 + 128], ones_row[0:1, :],
        start=False, stop=True,
    )


def _body(tc, nc, xqT, xkT, xvT, out, ones_row, sel64, rden, wq_sb, wk_sb,
          wv_sb, wo_sb, bq_sb, bk_sb, bv_sb, kT_sb, qT_sb, ctxT_sb, v_sb,
          xpool, epool, bpool, opool, mm_ps, sc_ps, ctx_ps, bc_ps):
    # ---- phase 1a: K^T = WkT.T @ XkT (+ bk outer ones) ----
    for qc in range(NQC if _on("kv") else 0):
        xk = []
        for kt in range(KT_E):
            t = xpool.tile([128, 512], F32R, tag="xchunk", name=f"xk_{qc}_{kt}")
            nc.sync.dma_start(t[:], xkT[kt * 128:(kt + 1) * 128, qc * 512:(qc + 1) * 512])
            xk.append(t)
        for dq in range(2):
            ps = mm_ps.tile([128, 512], F32, tag="sc", name=f"kps_{qc}_{dq}")
            _proj_chain(nc, ps, wk_sb, xk, dq, bk_sb, ones_row)
            nc.vector.tensor_copy(kT_sb[:, dq * S + qc * 512: dq * S + qc * 512 + 512], ps[:])

    # ---- phase 1b: V = XvT.T @ WvT (+ ones outer bv), scattered per head ----
    for qc in range(NQC if _on("kv") else 0):
        xv = []
        for kt in range(KT_E):
            t = xpool.tile([128, 512], F32R, tag="xchunk", name=f"xv_{qc}_{kt}")
            nc.sync.dma_start(t[:], xvT[kt * 128:(kt + 1) * 128, qc * 512:(qc + 1) * 512])
            xv.append(t)
        for sti in range(4):
            st = qc * 4 + sti
            ps = mm_ps.tile([128, 256], F32, tag="sc", name=f"vps_{st}")
            nc.tensor.matmul(
                ps[:], xv[0][:, sti * 128: sti * 128 + 128], wv_sb[:, 0:DQ],
                start=True, stop=False,
            )
            for kt in range(1, KT_E):
                if "v" in _SPLIT:
                    for b in (0, 64):
                        nc.tensor.matmul(
                            ps[:], xv[kt][b:b + 64, sti * 128: sti * 128 + 128],
                            wv_sb[b:b + 64, kt * DQ: kt * DQ + DQ],
                            start=False, stop=False,
                        )
                else:
                    nc.tensor.matmul(
                        ps[:], xv[kt][:, sti * 128: sti * 128 + 128],
                        wv_sb[:, kt * DQ: kt * DQ + DQ],
                        start=False, stop=False,
                    )
            nc.tensor.matmul(
                ps[:], ones_row[0:1, 0:128], bv_sb[0:1, :],
                start=False, stop=True,
            )
            dst = v_sb[:, st * (HPC * 65): (st + 1) * (HPC * 65)]
            nc.vector.tensor_copy(
                dst.rearrange("p (h m) -> p h m", h=HPC)[:, :, 0:64],
                ps[:].rearrange("p (h m) -> p h m", h=HPC),
            )

    # ---- phase 2: per query chunk: Q^T slice, attention, out projection ----
    for qc in range(NQC):
        xq = []
        for kt in range(KT_E if _on("q") else 0):
            t = xpool.tile([128, 512], F32R, tag="xchunk", name=f"xq_{qc}_{kt}")
            nc.sync.dma_start(t[:], xqT[kt * 128:(kt + 1) * 128, qc * 512:(qc + 1) * 512])
            xq.append(t)
        for dq in range(2 if _on("q") else 0):
            ps = mm_ps.tile([128, 512], F32, tag="sc", name=f"qps_{qc}_{dq}")
            _proj_chain(nc, ps, wq_sb, xq, dq, bq_sb, ones_row)
            nc.vector.tensor_copy(qT_sb[:, dq * S + qc * 512: dq * S + qc * 512 + 512], ps[:])

        # attention, two heads at a time (row-tiled K=64 matmuls can overlap)
        attn_on = _on("attn") or _on("attn_sc") or _on("attn_exp") or _on("attn_ctx")
        do_exp = _on("attn") or _on("attn_exp") or _on("attn_ctx")
        do_ctx = _on("attn") or _on("attn_ctx")
        do_norm = _on("attn")
        for hp in range(HPC // 2 if attn_on else 0):
            # per head: two homogeneous accumulation chains, one per row group
            # (upper 64 k-rows -> bank A at tile row 0, lower -> bank B at row
            # 64); they execute concurrently in the PE array.  Combined at
            # normalize time.
            cps = [
                [
                    ctx_ps.tile([65, 512], F32, tag="ctx", name=f"ctx_{qc}_{hp}_{i}_{half}")
                    for half in range(2)
                ]
                for i in range(2)
            ]

            def ctx_mms(es, kt):
                for hi in range(2):
                    h = hp * 2 + hi
                    rhs = es[hi][:]
                    vcol = kt * (HPC * 65) + h * 65
                    for half, b in enumerate((0, 64)):
                        nc.tensor.matmul(
                            cps[hi][half][:], v_sb[b:b + 64, vcol: vcol + 65],
                            rhs[b:b + 64, :],
                            start=(kt == 0), stop=(kt == NST - 1),
                        )

            prev = None
            for kt in range(NST):
                es = []
                sc = sc_ps.tile([128, 1024], F32, tag="sc", name=f"sc_{qc}_{hp}_{kt}")
                for hi in range(2):
                    base = 64 * hi
                    blk = hp * S
                    nc.tensor.matmul(
                        sc[:, hi * 512: hi * 512 + 512],
                        kT_sb[base:base + 64, blk + kt * 128: blk + kt * 128 + 128],
                        qT_sb[base:base + 64, blk + qc * 512: blk + qc * 512 + 512],
                        start=True, stop=True,
                    )
                if do_exp:
                    e = epool.tile([128, 1024], F32R, tag="e", name=f"e_{qc}_{hp}_{kt}")
                    nc.scalar.activation(e[:], sc[:], EXP, scale=0.125)
                    es = [e[:, 0:512], e[:, 512:1024]]
                if do_ctx:
                    if prev is not None:
                        ctx_mms(*prev)
                    prev = (es, kt)
            if do_ctx:
                ctx_mms(*prev)
            for hi in range(2 if do_norm else 0):
                # combine the two half-chains, broadcast the raw denominator to
                # 64 partitions via one-hot matmul, then reciprocal + multiply
                # (normalizes ctx on the way out of PSUM)
                tmpa = bpool.tile([65, 512], F32, tag="tmpa", name=f"tmpa_{qc}_{hp}_{hi}")
                nc.scalar.copy(tmpa[:], cps[hi][0][:])
                tmp = bpool.tile([65, 512], F32, tag="tmp", name=f"tmp_{qc}_{hp}_{hi}")
                nc.vector.tensor_add(tmp[:], cps[hi][1][:], tmpa[:])
                nc.vector.tensor_copy(rden[64:65, :], tmp[64:65, :])
                bps = bc_ps.tile([64, 512], F32, tag="sc", name=f"bc_{qc}_{hp}_{hi}")
                nc.tensor.matmul(bps[:], sel64[:], rden[:], start=True, stop=True)
                brec = bpool.tile([64, 512], F32, tag="br", name=f"br_{qc}_{hp}_{hi}")
                nc.vector.reciprocal(brec[:], bps[:])
                nc.vector.tensor_mul(
                    ctxT_sb[64 * hi: 64 * hi + 64, hp * S + qc * 512: hp * S + qc * 512 + 512],
                    tmp[0:64, :],
                    brec[:],
                )

        # out projection for this chunk's 4 query tiles
        for qt4 in range(4 if _on("out") else 0):
            qt = qc * 4 + qt4
            ot = opool.tile([128, EMB], F32, tag="o", name=f"ot_{qt}")
            for fc in range(2):
                ps = mm_ps.tile([128, 512], F32, tag="sc", name=f"ops_{qt}_{fc}")
                nc.tensor.matmul(
                    ps[:],
                    ctxT_sb[:, qt * 128: qt * 128 + 128],
                    wo_sb[:, fc * 512: fc * 512 + 512],
                    start=True, stop=False,
                )
                if "out" in _SPLIT:
                    for b in (0, 64):
                        nc.tensor.matmul(
                            ps[:],
                            ctxT_sb[b:b + 64, S + qt * 128: S + qt * 128 + 128],
                            wo_sb[b:b + 64, EMB + fc * 512: EMB + fc * 512 + 512],
                            start=False, stop=(b == 64),
                        )
                else:
                    nc.tensor.matmul(
                        ps[:],
                        ctxT_sb[:, S + qt * 128: S + qt * 128 + 128],
                        wo_sb[:, EMB + fc * 512: EMB + fc * 512 + 512],
                        start=False, stop=True,
                    )
                nc.vector.tensor_copy(ot[:, fc * 512: fc * 512 + 512], ps[:])
            nc.sync.dma_start(out[qt * 128:(qt + 1) * 128, :], ot[:])


def _build_nc(bench_iters=None):
    from contextlib import ExitStack

    nc = bacc.Bacc("TRN2", target_bir_lowering=False, debug=False, num_devices=NCORES)
    xqT = nc.dram_tensor("xqT", [EMB, S], F32R, kind="ExternalInput").ap()
    xkT = nc.dram_tensor("xkT", [EMB, S], F32R, kind="ExternalInput").ap()
    xvT = nc.dram_tensor("xvT", [EMB, S], F32R, kind="ExternalInput").ap()
    wqT = nc.dram_tensor("wqT", [EMB, DQ], F32R, kind="ExternalInput").ap()
    wkT = nc.dram_tensor("wkT", [EMB, DQ], F32R, kind="ExternalInput").ap()
    wvT = nc.dram_tensor("wvT", [EMB, DQ], F32R, kind="ExternalInput").ap()
    woT = nc.dram_tensor("woT", [DQ, EMB], F32R, kind="ExternalInput").ap()
    bq = nc.dram_tensor("bq", [1, DQ], F32R, kind="ExternalInput").ap()
    bk = nc.dram_tensor("bk", [1, DQ], F32R, kind="ExternalInput").ap()
    bv = nc.dram_tensor("bv", [1, DQ], F32R, kind="ExternalInput").ap()
    out = nc.dram_tensor("out", [S, EMB], F32, kind="ExternalOutput").ap()

    with ExitStack() as ctx:
        tc = ctx.enter_context(tile.TileContext(nc))
        _mha(ctx, tc, xqT, xkT, xvT, wqT, wkT, wvT, woT, bq, bk, bv, out,
             bench_iters=bench_iters)
    nc.compile()
    return nc


def kernel(query, key, value, Wq, bq, Wk, bk, Wv, bv, Wo, bo):
    global _NC, LAST_RESULT
    query, key, value, Wq, bq, Wk, bk, Wv, bv, Wo, bo = (
        np.asarray(a, dtype=np.float32)
        for a in (query, key, value, Wq, bq, Wk, bk, Wv, bv, Wo, bo)
    )
    if _NC is None:
        _NC = _build_nc()

    in_maps = []
    for c in range(NCORES):
        b, g = divmod(c, 4)
        rows = slice(g * DQ, (g + 1) * DQ)
        in_maps.append({
            "xqT": np.ascontiguousarray(query[b].T),
            "xkT": np.ascontiguousarray(key[b].T),
            "xvT": np.ascontiguousarray(value[b].T),
            "wqT": np.ascontiguousarray(Wq[rows].T),
            "wkT": np.ascontiguousarray(Wk[rows].T),
            "wvT": np.ascontiguousarray(Wv[rows].T),
            "woT": np.ascontiguousarray(Wo[:, rows].T),
            "bq": np.ascontiguousarray(bq[rows][None, :]),
            "bk": np.ascontiguousarray(bk[rows][None, :]),
            "bv": np.ascontiguousarray(bv[rows][None, :]),
        })

    res = bass_utils.run_bass_kernel_spmd(
        _NC, in_maps, core_ids=list(range(NCORES)), trace=TRACE
    )
    LAST_RESULT = res

    out = np.zeros((B, S, EMB), np.float32)
    for c in range(NCORES):
        out[c // 4] += res.results[c]["out"]
    out += bo[None, None, :]
    return out


# revision 49
# speedup vs baseline: 1.0522x; 1.0522x over previous
"""Multi-head attention (B=2, S=2048, EMB=1024, 16 heads) on 8 Trainium2 cores.

Sharding: core c -> batch c//4, head-group c%4 (4 heads = 256 projection dims).
Each core computes Q/K/V projections for its head group in transposed layout
(Q^T, K^T with head-dim on partitions; V natural), attention without max
subtraction (scores ~ N(0,1), exp never overflows in fp32), the softmax
denominator via a ones-column appended to V (free inside the ctx matmul), and
a row-parallel partial of the output projection.  The host sums the 4 partials
per batch and adds the output bias (the all-reduce of the row-parallel fc_out
is done at unshard time; no device collectives needed).

All matmuls run as float32r (full-rate on the PE at N>=256).  The BIR verifier
requires fp32r matmul operands to be *produced* as fp32r, so every
matmul-feeding tensor is declared fp32r end-to-end (same 4-byte layout as
fp32; host passes float32 arrays).
"""

import numpy as np

import concourse.tile as tile
from concourse import bacc, mybir
from concourse import bass_utils

EMB = 1024
S = 2048
B = 2
HPC = 4            # heads per core
DQ = HPC * 64      # 256 projection dims per core
NCORES = 8

F32 = mybir.dt.float32
F32R = mybir.dt.float32r
EXP = mybir.ActivationFunctionType.Exp

KT_E = EMB // 128  # 8 contraction tiles over EMB
NQC = S // 512     # 4 query chunks
NST = S // 128     # 16 sequence tiles

_NC = None
TRACE = False
LAST_RESULT = None
_ABLATE = None  # None = all phases; else subset of {"kv", "q", "attn", "out"}
_CTX_PROBE = None
_SPLIT = set()  # heterogeneous in-group splits are rejected by HW; keep off


def _on(phase):
    return _ABLATE is None or phase in _ABLATE


def _mha(ctx, tc, xqT, xkT, xvT, wqT, wkT, wvT, woT, bq, bk, bv, out, bench_iters=None):
    nc = tc.nc

    cpool = ctx.enter_context(tc.tile_pool(name="const", bufs=1))
    xpool = ctx.enter_context(tc.tile_pool(name="xin", bufs=16))
    epool = ctx.enter_context(tc.tile_pool(name="exp", bufs=6))
    bpool = ctx.enter_context(tc.tile_pool(name="bcsb", bufs=2))
    opool = ctx.enter_context(tc.tile_pool(name="osb", bufs=3))
    sc_ps = ctx.enter_context(tc.tile_pool(name="scps", bufs=4, space="PSUM"))
    ctx_ps = ctx.enter_context(tc.tile_pool(name="ctxps", bufs=4, space="PSUM"))
    mm_ps = sc_ps
    bc_ps = sc_ps

    # ---- persistent SBUF tensors ----
    ones_row = cpool.tile([1, 512], F32R)
    nc.vector.memset(ones_row[:].bitcast(F32), 1.0)
    sel64 = cpool.tile([65, 64], F32R)          # one-hot: row 64 -> all cols
    nc.vector.memset(sel64[:].bitcast(F32), 0.0)
    nc.vector.memset(sel64[64:65, :].bitcast(F32), 1.0)
    rden = cpool.tile([65, 512], F32R)          # row 64 = denom, rows 0..63 zero
    nc.vector.memset(rden[0:64, :].bitcast(F32), 0.0)

    wq_sb = cpool.tile([128, KT_E * DQ], F32R)  # [128, 2048]: wq_sb[p, n*256+m] = WqT[n*128+p, m]
    wk_sb = cpool.tile([128, KT_E * DQ], F32R)
    wv_sb = cpool.tile([128, KT_E * DQ], F32R)
    for sb, src in ((wq_sb, wqT), (wk_sb, wkT), (wv_sb, wvT)):
        nc.sync.dma_start(
            sb[:].rearrange("p (n m) -> p n m", n=KT_E),
            src.rearrange("(n p) m -> p n m", p=128),
        )
    wo_sb = cpool.tile([128, 2 * EMB], F32R)    # wo_sb[p, n*1024+f] = WoT[n*128+p, f]
    nc.sync.dma_start(
        wo_sb[:].rearrange("p (n m) -> p n m", n=2),
        woT.rearrange("(n p) m -> p n m", p=128),
    )
    bq_sb = cpool.tile([1, DQ], F32R)
    bk_sb = cpool.tile([1, DQ], F32R)
    bv_sb = cpool.tile([1, DQ], F32R)
    for sb, src in ((bq_sb, bq), (bk_sb, bk), (bv_sb, bv)):
        nc.sync.dma_start(sb[:], src[:])

    # results of phase 1/2 kept resident
    kT_sb = cpool.tile([128, 2 * S], F32R)      # [dq-block 2][s 2048]
    qT_sb = cpool.tile([128, 2 * S], F32R)
    ctxT_sb = cpool.tile([128, 2 * S], F32R)
    v_sb = cpool.tile([128, NST * (HPC * 65)], F32R)  # per s-tile: 4 heads x (64 V + ones col)
    nc.vector.memset(
        v_sb[:].bitcast(F32).rearrange("p (t h m) -> p t h m", t=NST, h=HPC)[:, :, :, 64:65],
        1.0,
    )

    if _ABLATE:
        # ablation: un-computed persistent tensors need defined contents
        for t in (kT_sb, qT_sb, ctxT_sb, v_sb):
            nc.vector.memset(t[:].bitcast(F32), 0.001)
        nc.vector.memset(rden[64:65, :].bitcast(F32), 1.0)

    def body():
        _body(tc, nc, xqT, xkT, xvT, out, ones_row, sel64, rden, wq_sb, wk_sb,
              wv_sb, wo_sb, bq_sb, bk_sb, bv_sb, kT_sb, qT_sb, ctxT_sb, v_sb,
              xpool, epool, bpool, opool, mm_ps, sc_ps, ctx_ps, bc_ps)

    if bench_iters:
        hints = (
            mybir.EngineType.PE,
            mybir.EngineType.Activation,
            mybir.EngineType.DVE,
            mybir.EngineType.SP,
            mybir.EngineType.Pool,
        )
        with tc.For_i(0, bench_iters, 1, hint_engines=hints):
            body()
    else:
        body()


def _proj_chain(nc, ps, w_sb, xs, dq, b_sb, ones_row):
    """Q/K projection chain into psum ps: kt=0 full K=128 (group start, marks
    every element written), then K=64 halves on alternating row groups (they
    overlap in the PE array), bias last."""
    col = lambda kt: kt * DQ + dq * 128
    nc.tensor.matmul(ps[:], w_sb[:, col(0): col(0) + 128], xs[0][:],
                     start=True, stop=False)
    for kt in range(1, KT_E):
        if "proj" in _SPLIT:
            for b in (0, 64):
                nc.tensor.matmul(
                    ps[:], w_sb[b:b + 64, col(kt): col(kt) + 128], xs[kt][b:b + 64, :],
                    start=False, stop=False,
                )
        else:
            nc.tensor.matmul(
                ps[:], w_sb[:, col(kt): col(kt) + 128], xs[kt][:],
                start=False, stop=False,
            )
    nc.tensor.matmul(
        ps[:], b_sb[0:1, dq * 128: dq * 128 + 128], ones_row[0:1, :],
        start=False, stop=True,
    )


def _body(tc, nc, xqT, xkT, xvT, out, ones_row, sel64, rden, wq_sb, wk_sb,
          wv_sb, wo_sb, bq_sb, bk_sb, bv_sb, kT_sb, qT_sb, ctxT_sb, v_sb,
          xpool, epool, bpool, opool, mm_ps, sc_ps, ctx_ps, bc_ps):
    # ---- phase 1a: K^T = WkT.T @ XkT (+ bk outer ones) ----
    for qc in range(NQC if _on("kv") else 0):
        xk = []
        for kt in range(KT_E):
            t = xpool.tile([128, 512], F32R, tag="xchunk", name=f"xk_{qc}_{kt}")
            nc.sync.dma_start(t[:], xkT[kt * 128:(kt + 1) * 128, qc * 512:(qc + 1) * 512])
            xk.append(t)
        for dq in range(2):
            ps = mm_ps.tile([128, 512], F32, tag="sc", name=f"kps_{qc}_{dq}")
            _proj_chain(nc, ps, wk_sb, xk, dq, bk_sb, ones_row)
            nc.vector.tensor_copy(kT_sb[:, dq * S + qc * 512: dq * S + qc * 512 + 512], ps[:])

    # ---- phase 1b: V = XvT.T @ WvT (+ ones outer bv), scattered per head ----
    for qc in range(NQC if _on("kv") else 0):
        xv = []
        for kt in range(KT_E):
            t = xpool.tile([128, 512], F32R, tag="xchunk", name=f"xv_{qc}_{kt}")
            nc.sync.dma_start(t[:], xvT[kt * 128:(kt + 1) * 128, qc * 512:(qc + 1) * 512])
            xv.append(t)
        for sti in range(4):
            st = qc * 4 + sti
            ps = mm_ps.tile([128, 256], F32, tag="sc", name=f"vps_{st}")
            nc.tensor.matmul(
                ps[:], xv[0][:, sti * 128: sti * 128 + 128], wv_sb[:, 0:DQ],
                start=True, stop=False,
            )
            for kt in range(1, KT_E):
                if "v" in _SPLIT:
                    for b in (0, 64):
                        nc.tensor.matmul(
                            ps[:], xv[kt][b:b + 64, sti * 128: sti * 128 + 128],
                            wv_sb[b:b + 64, kt * DQ: kt * DQ + DQ],
                            start=False, stop=False,
                        )
                else:
                    nc.tensor.matmul(
                        ps[:], xv[kt][:, sti * 128: sti * 128 + 128],
                        wv_sb[:, kt * DQ: kt * DQ + DQ],
                        start=False, stop=False,
                    )
            nc.tensor.matmul(
                ps[:], ones_row[0:1, 0:128], bv_sb[0:1, :],
                start=False, stop=True,
            )
            dst = v_sb[:, st * (HPC * 65): (st + 1) * (HPC * 65)]
            nc.vector.tensor_copy(
                dst.rearrange("p (h m) -> p h m", h=HPC)[:, :, 0:64],
                ps[:].rearrange("p (h m) -> p h m", h=HPC),
            )

    # ---- phase 2: per query chunk: Q^T slice, attention, out projection ----
    for qc in range(NQC):
        xq = []
        for kt in range(KT_E if _on("q") else 0):
            t = xpool.tile([128, 512], F32R, tag="xchunk", name=f"xq_{qc}_{kt}")
            nc.sync.dma_start(t[:], xqT[kt * 128:(kt + 1) * 128, qc * 512:(qc + 1) * 512])
            xq.append(t)
        for dq in range(2 if _on("q") else 0):
            ps = mm_ps.tile([128, 512], F32, tag="sc", name=f"qps_{qc}_{dq}")
            _proj_chain(nc, ps, wq_sb, xq, dq, bq_sb, ones_row)
            nc.vector.tensor_copy(qT_sb[:, dq * S + qc * 512: dq * S + qc * 512 + 512], ps[:])

        # attention, two heads at a time (row-tiled K=64 matmuls can overlap)
        attn_on = _on("attn") or _on("attn_sc") or _on("attn_exp") or _on("attn_ctx")
        do_exp = _on("attn") or _on("attn_exp") or _on("attn_ctx")
        do_ctx = _on("attn") or _on("attn_ctx")
        do_norm = _on("attn")
        for hp in range(HPC // 2 if attn_on else 0):
            # per head: two homogeneous accumulation chains, one per row group
            # (upper 64 k-rows -> bank A at tile row 0, lower -> bank B at row
            # 64); they execute concurrently in the PE array.  Combined at
            # normalize time.
            cps = [
                [
                    ctx_ps.tile([65, 512], F32, tag="ctx", name=f"ctx_{qc}_{hp}_{i}_{half}")
                    for half in range(2)
                ]
                for i in range(2)
            ]

            def ctx_mms(es, kt):
                for hi in range(2):
                    h = hp * 2 + hi
                    rhs = es[hi][:]
                    vcol = kt * (HPC * 65) + h * 65
                    for half, b in enumerate((0, 64)):
                        nc.tensor.matmul(
                            cps[hi][half][:], v_sb[b:b + 64, vcol: vcol + 65],
                            rhs[b:b + 64, :],
                            start=(kt == 0), stop=(kt == NST - 1),
                        )

            prev = None
            for kt in range(NST):
                es = []
                for hi in range(2):
                    base = 64 * hi
                    blk = hp * S
                    sc = sc_ps.tile([128, 512], F32, tag="sc", name=f"sc_{qc}_{hp}_{kt}_{hi}")
                    nc.tensor.matmul(
                        sc[:],
                        kT_sb[base:base + 64, blk + kt * 128: blk + kt * 128 + 128],
                        qT_sb[base:base + 64, blk + qc * 512: blk + qc * 512 + 512],
                        start=True, stop=True,
                    )
                    if do_exp:
                        e = epool.tile([128, 512], F32R, tag="e", name=f"e_{qc}_{hp}_{kt}_{hi}")
                        nc.scalar.activation(e[:], sc[:], EXP, scale=0.125)
                        es.append(e)
                if do_ctx:
                    if prev is not None:
                        ctx_mms(*prev)
                    prev = (es, kt)
            if do_ctx:
                ctx_mms(*prev)
            for hi in range(2 if do_norm else 0):
                # combine the two half-chains, broadcast the raw denominator to
                # 64 partitions via one-hot matmul, then reciprocal + multiply
                # (normalizes ctx on the way out of PSUM)
                tmpa = bpool.tile([65, 512], F32, tag="tmpa", name=f"tmpa_{qc}_{hp}_{hi}")
                nc.scalar.copy(tmpa[:], cps[hi][0][:])
                tmp = bpool.tile([65, 512], F32, tag="tmp", name=f"tmp_{qc}_{hp}_{hi}")
                nc.vector.tensor_add(tmp[:], cps[hi][1][:], tmpa[:])
                nc.vector.tensor_copy(rden[64:65, :], tmp[64:65, :])
                bps = bc_ps.tile([64, 512], F32, tag="sc", name=f"bc_{qc}_{hp}_{hi}")
                nc.tensor.matmul(bps[:], sel64[:], rden[:], start=True, stop=True)
                brec = bpool.tile([64, 512], F32, tag="br", name=f"br_{qc}_{hp}_{hi}")
                nc.vector.reciprocal(brec[:], bps[:])
                nc.vector.tensor_mul(
                    ctxT_sb[64 * hi: 64 * hi + 64, hp * S + qc * 512: hp * S + qc * 512 + 512],
                    tmp[0:64, :],
                    brec[:],
                )

        # out projection for this chunk's 4 query tiles
        for qt4 in range(4 if _on("out") else 0):
            qt = qc * 4 + qt4
            ot = opool.tile([128, EMB], F32, tag="o", name=f"ot_{qt}")
            for fc in range(2):
                ps = mm_ps.tile([128, 512], F32, tag="sc", name=f"ops_{qt}_{fc}")
                nc.tensor.matmul(
                    ps[:],
                    ctxT_sb[:, qt * 128: qt * 128 + 128],
                    wo_sb[:, fc * 512: fc * 512 + 512],
                    start=True, stop=False,
                )
                if "out" in _SPLIT:
                    for b in (0, 64):
                        nc.tensor.matmul(
                            ps[:],
                            ctxT_sb[b:b + 64, S + qt * 128: S + qt * 128 + 128],
                            wo_sb[b:b + 64, EMB + fc * 512: EMB + fc * 512 + 512],
                            start=False, stop=(b == 64),
                        )
                else:
                    nc.tensor.matmul(
                        ps[:],
                        ctxT_sb[:, S + qt * 128: S + qt * 128 + 128],
                        wo_sb[:, EMB + fc * 512: EMB + fc * 512 + 512],
                        start=False, stop=True,
                    )
                nc.vector.tensor_copy(ot[:, fc * 512: fc * 512 + 512], ps[:])
            nc.sync.dma_start(out[qt * 128:(qt + 1) * 128, :], ot[:])


def _build_nc(bench_iters=None):
    from contextlib import ExitStack

    nc = bacc.Bacc("TRN2", target_bir_lowering=False, debug=False, num_devices=NCORES)
    xqT = nc.dram_tensor("xqT", [EMB, S], F32R, kind="ExternalInput").ap()
    xkT = nc.dram_tensor("xkT", [EMB, S], F32R, kind="ExternalInput").ap()
    xvT = nc.dram_tensor("xvT", [EMB, S], F32R, kind="ExternalInput").ap()
    wqT = nc.dram_tensor("wqT", [EMB, DQ], F32R, kind="ExternalInput").ap()
    wkT = nc.dram_tensor("wkT", [EMB, DQ], F32R, kind="ExternalInput").ap()
    wvT = nc.dram_tensor("wvT", [EMB, DQ], F32R, kind="ExternalInput").ap()
    woT = nc.dram_tensor("woT", [DQ, EMB], F32R, kind="ExternalInput").ap()
    bq = nc.dram_tensor("bq", [1, DQ], F32R, kind="ExternalInput").ap()
    bk = nc.dram_tensor("bk", [1, DQ], F32R, kind="ExternalInput").ap()
    bv = nc.dram_tensor("bv", [1, DQ], F32R, kind="ExternalInput").ap()
    out = nc.dram_tensor("out", [S, EMB], F32, kind="ExternalOutput").ap()

    with ExitStack() as ctx:
        tc = ctx.enter_context(tile.TileContext(nc))
        _mha(ctx, tc, xqT, xkT, xvT, wqT, wkT, wvT, woT, bq, bk, bv, out,
             bench_iters=bench_iters)
    nc.compile()
    return nc


def kernel(query, key, value, Wq, bq, Wk, bk, Wv, bv, Wo, bo):
    global _NC, LAST_RESULT
    query, key, value, Wq, bq, Wk, bk, Wv, bv, Wo, bo = (
        np.asarray(a, dtype=np.float32)
        for a in (query, key, value, Wq, bq, Wk, bk, Wv, bv, Wo, bo)
    )
    if _NC is None:
        _NC = _build_nc()

    in_maps = []
    for c in range(NCORES):
        b, g = divmod(c, 4)
        rows = slice(g * DQ, (g + 1) * DQ)
        in_maps.append({
            "xqT": np.ascontiguousarray(query[b].T),
            "xkT": np.ascontiguousarray(key[b].T),
            "xvT": np.ascontiguousarray(value[b].T),
            "wqT": np.ascontiguousarray(Wq[rows].T),
            "wkT": np.ascontiguousarray(Wk[rows].T),
            "wvT": np.ascontiguousarray(Wv[rows].T),
            "woT": np.ascontiguousarray(Wo[:, rows].T),
            "bq": np.ascontiguousarray(bq[rows][None, :]),
            "bk": np.ascontiguousarray(bk[rows][None, :]),
            "bv": np.ascontiguousarray(bv[rows][None, :]),
        })

    res = bass_utils.run_bass_kernel_spmd(
        _NC, in_maps, core_ids=list(range(NCORES)), trace=TRACE
    )
    LAST_RESULT = res

    out = np.zeros((B, S, EMB), np.float32)
    for c in range(NCORES):
        out[c // 4] += res.results[c]["out"]
    out += bo[None, None, :]
    return out


# revision 50
# speedup vs baseline: 1.0684x; 1.0154x over previous
"""Multi-head attention (B=2, S=2048, EMB=1024, 16 heads) on 8 Trainium2 cores.

Sharding: core c -> batch c//4, head-group c%4 (4 heads = 256 projection dims).
Each core computes Q/K/V projections for its head group in transposed layout
(Q^T, K^T with head-dim on partitions; V natural), attention without max
subtraction (scores ~ N(0,1), exp never overflows in fp32), the softmax
denominator via a ones-column appended to V (free inside the ctx matmul), and
a row-parallel partial of the output projection.  The host sums the 4 partials
per batch and adds the output bias (the all-reduce of the row-parallel fc_out
is done at unshard time; no device collectives needed).

All matmuls run as float32r (full-rate on the PE at N>=256).  The BIR verifier
requires fp32r matmul operands to be *produced* as fp32r, so every
matmul-feeding tensor is declared fp32r end-to-end (same 4-byte layout as
fp32; host passes float32 arrays).
"""

import numpy as np

import concourse.tile as tile
from concourse import bacc, mybir
from concourse import bass_utils

EMB = 1024
S = 2048
B = 2
HPC = 4            # heads per core
DQ = HPC * 64      # 256 projection dims per core
NCORES = 8

F32 = mybir.dt.float32
F32R = mybir.dt.float32r
EXP = mybir.ActivationFunctionType.Exp

KT_E = EMB // 128  # 8 contraction tiles over EMB
NQC = S // 512     # 4 query chunks
NST = S // 128     # 16 sequence tiles

_NC = None
TRACE = False
LAST_RESULT = None
_ABLATE = None  # None = all phases; else subset of {"kv", "q", "attn", "out"}
_CTX_PROBE = None
_SPLIT = set()  # heterogeneous in-group splits are rejected by HW; keep off


def _on(phase):
    return _ABLATE is None or phase in _ABLATE


def _mha(ctx, tc, xqT, xkT, xvT, wqT, wkT, wvT, woT, bq, bk, bv, out, bench_iters=None):
    nc = tc.nc

    cpool = ctx.enter_context(tc.tile_pool(name="const", bufs=1))
    xpool = ctx.enter_context(tc.tile_pool(name="xin", bufs=16))
    epool = ctx.enter_context(tc.tile_pool(name="exp", bufs=6))
    bpool = ctx.enter_context(tc.tile_pool(name="bcsb", bufs=2))
    opool = ctx.enter_context(tc.tile_pool(name="osb", bufs=3))
    sc_ps = ctx.enter_context(tc.tile_pool(name="scps", bufs=4, space="PSUM"))
    ctx_ps = ctx.enter_context(tc.tile_pool(name="ctxps", bufs=4, space="PSUM"))
    mm_ps = sc_ps
    bc_ps = sc_ps

    # ---- persistent SBUF tensors ----
    ones_row = cpool.tile([1, 512], F32R)
    nc.vector.memset(ones_row[:].bitcast(F32), 1.0)
    sel64 = cpool.tile([65, 64], F32R)          # one-hot: row 64 -> all cols
    nc.vector.memset(sel64[:].bitcast(F32), 0.0)
    nc.vector.memset(sel64[64:65, :].bitcast(F32), 1.0)
    rden = cpool.tile([65, 512], F32R)          # row 64 = denom, rows 0..63 zero
    nc.vector.memset(rden[0:64, :].bitcast(F32), 0.0)

    wq_sb = cpool.tile([128, KT_E * DQ], F32R)  # [128, 2048]: wq_sb[p, n*256+m] = WqT[n*128+p, m]
    wk_sb = cpool.tile([128, KT_E * DQ], F32R)
    wv_sb = cpool.tile([128, KT_E * DQ], F32R)
    for sb, src in ((wq_sb, wqT), (wk_sb, wkT), (wv_sb, wvT)):
        nc.sync.dma_start(
            sb[:].rearrange("p (n m) -> p n m", n=KT_E),
            src.rearrange("(n p) m -> p n m", p=128),
        )
    wo_sb = cpool.tile([128, 2 * EMB], F32R)    # wo_sb[p, n*1024+f] = WoT[n*128+p, f]
    nc.sync.dma_start(
        wo_sb[:].rearrange("p (n m) -> p n m", n=2),
        woT.rearrange("(n p) m -> p n m", p=128),
    )
    bq_sb = cpool.tile([1, DQ], F32R)
    bk_sb = cpool.tile([1, DQ], F32R)
    bv_sb = cpool.tile([1, DQ], F32R)
    for sb, src in ((bq_sb, bq), (bk_sb, bk), (bv_sb, bv)):
        nc.sync.dma_start(sb[:], src[:])

    # results of phase 1/2 kept resident
    kT_sb = cpool.tile([128, 2 * S], F32R)      # [dq-block 2][s 2048]
    qT_sb = cpool.tile([128, 2 * S], F32R)
    ctxT_sb = cpool.tile([128, 2 * S], F32R)
    v_sb = cpool.tile([128, NST * (HPC * 65)], F32R)  # per s-tile: 4 heads x (64 V + ones col)
    nc.vector.memset(
        v_sb[:].bitcast(F32).rearrange("p (t h m) -> p t h m", t=NST, h=HPC)[:, :, :, 64:65],
        1.0,
    )

    if _ABLATE:
        # ablation: un-computed persistent tensors need defined contents
        for t in (kT_sb, qT_sb, ctxT_sb, v_sb):
            nc.vector.memset(t[:].bitcast(F32), 0.001)
        nc.vector.memset(rden[64:65, :].bitcast(F32), 1.0)

    def body():
        _body(tc, nc, xqT, xkT, xvT, out, ones_row, sel64, rden, wq_sb, wk_sb,
              wv_sb, wo_sb, bq_sb, bk_sb, bv_sb, kT_sb, qT_sb, ctxT_sb, v_sb,
              xpool, epool, bpool, opool, mm_ps, sc_ps, ctx_ps, bc_ps)

    if bench_iters:
        hints = (
            mybir.EngineType.PE,
            mybir.EngineType.Activation,
            mybir.EngineType.DVE,
            mybir.EngineType.SP,
            mybir.EngineType.Pool,
        )
        with tc.For_i(0, bench_iters, 1, hint_engines=hints):
            body()
    else:
        body()


def _proj_chain(nc, ps, w_sb, xs, dq, b_sb, ones_row):
    """Q/K projection chain into psum ps: kt=0 full K=128 (group start, marks
    every element written), then K=64 halves on alternating row groups (they
    overlap in the PE array), bias last."""
    col = lambda kt: kt * DQ + dq * 128
    nc.tensor.matmul(ps[:], w_sb[:, col(0): col(0) + 128], xs[0][:],
                     start=True, stop=False)
    for kt in range(1, KT_E):
        if "proj" in _SPLIT:
            for b in (0, 64):
                nc.tensor.matmul(
                    ps[:], w_sb[b:b + 64, col(kt): col(kt) + 128], xs[kt][b:b + 64, :],
                    start=False, stop=False,
                )
        else:
            nc.tensor.matmul(
                ps[:], w_sb[:, col(kt): col(kt) + 128], xs[kt][:],
                start=False, stop=False,
            )
    nc.tensor.matmul(
        ps[:], b_sb[0:1, dq * 128: dq * 128 + 128], ones_row[0:1, :],
        start=False, stop=True,
    )


def _body(tc, nc, xqT, xkT, xvT, out, ones_row, sel64, rden, wq_sb, wk_sb,
          wv_sb, wo_sb, bq_sb, bk_sb, bv_sb, kT_sb, qT_sb, ctxT_sb, v_sb,
          xpool, epool, bpool, opool, mm_ps, sc_ps, ctx_ps, bc_ps):
    # ---- phase 1a: K^T = WkT.T @ XkT (+ bk outer ones) ----
    for qc in range(NQC if _on("kv") else 0):
        xk = []
        for kt in range(KT_E):
            t = xpool.tile([128, 512], F32R, tag="xchunk", name=f"xk_{qc}_{kt}")
            nc.sync.dma_start(t[:], xkT[kt, qc])
            xk.append(t)
        for dq in range(2):
            ps = mm_ps.tile([128, 512], F32, tag="sc", name=f"kps_{qc}_{dq}")
            _proj_chain(nc, ps, wk_sb, xk, dq, bk_sb, ones_row)
            nc.vector.tensor_copy(kT_sb[:, dq * S + qc * 512: dq * S + qc * 512 + 512], ps[:])

    # ---- phase 1b: V = XvT.T @ WvT (+ ones outer bv), scattered per head ----
    for qc in range(NQC if _on("kv") else 0):
        xv = []
        for kt in range(KT_E):
            t = xpool.tile([128, 512], F32R, tag="xchunk", name=f"xv_{qc}_{kt}")
            nc.sync.dma_start(t[:], xvT[kt, qc])
            xv.append(t)
        for sti in range(4):
            st = qc * 4 + sti
            ps = mm_ps.tile([128, 256], F32, tag="sc", name=f"vps_{st}")
            nc.tensor.matmul(
                ps[:], xv[0][:, sti * 128: sti * 128 + 128], wv_sb[:, 0:DQ],
                start=True, stop=False,
            )
            for kt in range(1, KT_E):
                if "v" in _SPLIT:
                    for b in (0, 64):
                        nc.tensor.matmul(
                            ps[:], xv[kt][b:b + 64, sti * 128: sti * 128 + 128],
                            wv_sb[b:b + 64, kt * DQ: kt * DQ + DQ],
                            start=False, stop=False,
                        )
                else:
                    nc.tensor.matmul(
                        ps[:], xv[kt][:, sti * 128: sti * 128 + 128],
                        wv_sb[:, kt * DQ: kt * DQ + DQ],
                        start=False, stop=False,
                    )
            nc.tensor.matmul(
                ps[:], ones_row[0:1, 0:128], bv_sb[0:1, :],
                start=False, stop=True,
            )
            dst = v_sb[:, st * (HPC * 65): (st + 1) * (HPC * 65)]
            nc.vector.tensor_copy(
                dst.rearrange("p (h m) -> p h m", h=HPC)[:, :, 0:64],
                ps[:].rearrange("p (h m) -> p h m", h=HPC),
            )

    # ---- phase 2: per query chunk: Q^T slice, attention, out projection ----
    for qc in range(NQC):
        xq = []
        for kt in range(KT_E if _on("q") else 0):
            t = xpool.tile([128, 512], F32R, tag="xchunk", name=f"xq_{qc}_{kt}")
            nc.sync.dma_start(t[:], xqT[kt, qc])
            xq.append(t)
        for dq in range(2 if _on("q") else 0):
            ps = mm_ps.tile([128, 512], F32, tag="sc", name=f"qps_{qc}_{dq}")
            _proj_chain(nc, ps, wq_sb, xq, dq, bq_sb, ones_row)
            nc.vector.tensor_copy(qT_sb[:, dq * S + qc * 512: dq * S + qc * 512 + 512], ps[:])

        # attention, two heads at a time (row-tiled K=64 matmuls can overlap)
        attn_on = _on("attn") or _on("attn_sc") or _on("attn_exp") or _on("attn_ctx")
        do_exp = _on("attn") or _on("attn_exp") or _on("attn_ctx")
        do_ctx = _on("attn") or _on("attn_ctx")
        do_norm = _on("attn")
        for hp in range(HPC // 2 if attn_on else 0):
            # per head: two homogeneous accumulation chains, one per row group
            # (upper 64 k-rows -> bank A at tile row 0, lower -> bank B at row
            # 64); they execute concurrently in the PE array.  Combined at
            # normalize time.
            cps = [
                [
                    ctx_ps.tile([65, 512], F32, tag="ctx", name=f"ctx_{qc}_{hp}_{i}_{half}")
                    for half in range(2)
                ]
                for i in range(2)
            ]

            def ctx_mms(es, kt):
                for hi in range(2):
                    h = hp * 2 + hi
                    rhs = es[hi][:]
                    vcol = kt * (HPC * 65) + h * 65
                    for half, b in enumerate((0, 64)):
                        nc.tensor.matmul(
                            cps[hi][half][:], v_sb[b:b + 64, vcol: vcol + 65],
                            rhs[b:b + 64, :],
                            start=(kt == 0), stop=(kt == NST - 1),
                        )

            prev = None
            for kt in range(NST):
                es = []
                for hi in range(2):
                    base = 64 * hi
                    blk = hp * S
                    sc = sc_ps.tile([128, 512], F32, tag="sc", name=f"sc_{qc}_{hp}_{kt}_{hi}")
                    nc.tensor.matmul(
                        sc[:],
                        kT_sb[base:base + 64, blk + kt * 128: blk + kt * 128 + 128],
                        qT_sb[base:base + 64, blk + qc * 512: blk + qc * 512 + 512],
                        start=True, stop=True,
                    )
                    if do_exp:
                        e = epool.tile([128, 512], F32R, tag="e", name=f"e_{qc}_{hp}_{kt}_{hi}")
                        nc.scalar.activation(e[:], sc[:], EXP, scale=0.125)
                        es.append(e)
                if do_ctx:
                    if prev is not None:
                        ctx_mms(*prev)
                    prev = (es, kt)
            if do_ctx:
                ctx_mms(*prev)
            for hi in range(2 if do_norm else 0):
                # combine the two half-chains, broadcast the raw denominator to
                # 64 partitions via one-hot matmul, then reciprocal + multiply
                # (normalizes ctx on the way out of PSUM)
                tmpa = bpool.tile([65, 512], F32, tag="tmpa", name=f"tmpa_{qc}_{hp}_{hi}")
                nc.scalar.copy(tmpa[:], cps[hi][0][:])
                tmp = bpool.tile([65, 512], F32, tag="tmp", name=f"tmp_{qc}_{hp}_{hi}")
                nc.vector.tensor_add(tmp[:], cps[hi][1][:], tmpa[:])
                nc.vector.tensor_copy(rden[64:65, :], tmp[64:65, :])
                bps = bc_ps.tile([64, 512], F32, tag="sc", name=f"bc_{qc}_{hp}_{hi}")
                nc.tensor.matmul(bps[:], sel64[:], rden[:], start=True, stop=True)
                brec = bpool.tile([64, 512], F32, tag="br", name=f"br_{qc}_{hp}_{hi}")
                nc.vector.reciprocal(brec[:], bps[:])
                nc.vector.tensor_mul(
                    ctxT_sb[64 * hi: 64 * hi + 64, hp * S + qc * 512: hp * S + qc * 512 + 512],
                    tmp[0:64, :],
                    brec[:],
                )

        # out projection for this chunk's 4 query tiles
        for qt4 in range(4 if _on("out") else 0):
            qt = qc * 4 + qt4
            ot = opool.tile([128, EMB], F32, tag="o", name=f"ot_{qt}")
            for fc in range(2):
                ps = mm_ps.tile([128, 512], F32, tag="sc", name=f"ops_{qt}_{fc}")
                nc.tensor.matmul(
                    ps[:],
                    ctxT_sb[:, qt * 128: qt * 128 + 128],
                    wo_sb[:, fc * 512: fc * 512 + 512],
                    start=True, stop=False,
                )
                if "out" in _SPLIT:
                    for b in (0, 64):
                        nc.tensor.matmul(
                            ps[:],
                            ctxT_sb[b:b + 64, S + qt * 128: S + qt * 128 + 128],
                            wo_sb[b:b + 64, EMB + fc * 512: EMB + fc * 512 + 512],
                            start=False, stop=(b == 64),
                        )
                else:
                    nc.tensor.matmul(
                        ps[:],
                        ctxT_sb[:, S + qt * 128: S + qt * 128 + 128],
                        wo_sb[:, EMB + fc * 512: EMB + fc * 512 + 512],
                        start=False, stop=True,
                    )
                nc.vector.tensor_copy(ot[:, fc * 512: fc * 512 + 512], ps[:])
            nc.gpsimd.dma_start(out[qt * 128:(qt + 1) * 128, :], ot[:])


def _build_nc(bench_iters=None):
    from contextlib import ExitStack

    nc = bacc.Bacc("TRN2", target_bir_lowering=False, debug=False, num_devices=NCORES)
    xqT = nc.dram_tensor("xqT", [KT_E, NQC, 128, 512], F32R, kind="ExternalInput").ap()
    xkT = nc.dram_tensor("xkT", [KT_E, NQC, 128, 512], F32R, kind="ExternalInput").ap()
    xvT = nc.dram_tensor("xvT", [KT_E, NQC, 128, 512], F32R, kind="ExternalInput").ap()
    wqT = nc.dram_tensor("wqT", [EMB, DQ], F32R, kind="ExternalInput").ap()
    wkT = nc.dram_tensor("wkT", [EMB, DQ], F32R, kind="ExternalInput").ap()
    wvT = nc.dram_tensor("wvT", [EMB, DQ], F32R, kind="ExternalInput").ap()
    woT = nc.dram_tensor("woT", [DQ, EMB], F32R, kind="ExternalInput").ap()
    bq = nc.dram_tensor("bq", [1, DQ], F32R, kind="ExternalInput").ap()
    bk = nc.dram_tensor("bk", [1, DQ], F32R, kind="ExternalInput").ap()
    bv = nc.dram_tensor("bv", [1, DQ], F32R, kind="ExternalInput").ap()
    out = nc.dram_tensor("out", [S, EMB], F32, kind="ExternalOutput").ap()

    with ExitStack() as ctx:
        tc = ctx.enter_context(tile.TileContext(nc))
        _mha(ctx, tc, xqT, xkT, xvT, wqT, wkT, wvT, woT, bq, bk, bv, out,
             bench_iters=bench_iters)
    nc.compile()
    return nc


def _chunk_major(x):
    """[S, EMB] -> x.T chunked as [KT_E, NQC, 128, 512] (each chunk contiguous)."""
    xt = x.T  # [EMB, S]
    return np.ascontiguousarray(
        xt.reshape(KT_E, 128, NQC, 512).transpose(0, 2, 1, 3)
    )


def kernel(query, key, value, Wq, bq, Wk, bk, Wv, bv, Wo, bo):
    global _NC, LAST_RESULT
    query, key, value, Wq, bq, Wk, bk, Wv, bv, Wo, bo = (
        np.asarray(a, dtype=np.float32)
        for a in (query, key, value, Wq, bq, Wk, bk, Wv, bv, Wo, bo)
    )
    if _NC is None:
        _NC = _build_nc()

    in_maps = []
    for c in range(NCORES):
        b, g = divmod(c, 4)
        rows = slice(g * DQ, (g + 1) * DQ)
        in_maps.append({
            "xqT": _chunk_major(query[b]),
            "xkT": _chunk_major(key[b]),
            "xvT": _chunk_major(value[b]),
            "wqT": np.ascontiguousarray(Wq[rows].T),
            "wkT": np.ascontiguousarray(Wk[rows].T),
            "wvT": np.ascontiguousarray(Wv[rows].T),
            "woT": np.ascontiguousarray(Wo[:, rows].T),
            "bq": np.ascontiguousarray(bq[rows][None, :]),
            "bk": np.ascontiguousarray(bk[rows][None, :]),
            "bv": np.ascontiguousarray(bv[rows][None, :]),
        })

    res = bass_utils.run_bass_kernel_spmd(
        _NC, in_maps, core_ids=list(range(NCORES)), trace=TRACE
    )
    LAST_RESULT = res

    out = np.zeros((B, S, EMB), np.float32)
    for c in range(NCORES):
        out[c // 4] += res.results[c]["out"]
    out += bo[None, None, :]
    return out


# revision 51
# speedup vs baseline: 1.0758x; 1.0069x over previous
"""Multi-head attention (B=2, S=2048, EMB=1024, 16 heads) on 8 Trainium2 cores.

Sharding: core c -> batch c//4, head-group c%4 (4 heads = 256 projection dims).
Each core computes Q/K/V projections for its head group in transposed layout
(Q^T, K^T with head-dim on partitions; V natural), attention without max
subtraction (scores ~ N(0,1), exp never overflows in fp32), the softmax
denominator via a ones-column appended to V (free inside the ctx matmul), and
a row-parallel partial of the output projection.  The host sums the 4 partials
per batch and adds the output bias (the all-reduce of the row-parallel fc_out
is done at unshard time; no device collectives needed).

All matmuls run as float32r (full-rate on the PE at N>=256).  The BIR verifier
requires fp32r matmul operands to be *produced* as fp32r, so every
matmul-feeding tensor is declared fp32r end-to-end (same 4-byte layout as
fp32; host passes float32 arrays).
"""

import numpy as np

import concourse.tile as tile
from concourse import bacc, mybir
from concourse import bass_utils

EMB = 1024
S = 2048
B = 2
HPC = 4            # heads per core
DQ = HPC * 64      # 256 projection dims per core
NCORES = 8

F32 = mybir.dt.float32
F32R = mybir.dt.float32r
EXP = mybir.ActivationFunctionType.Exp

KT_E = EMB // 128  # 8 contraction tiles over EMB
NQC = S // 512     # 4 query chunks
NST = S // 128     # 16 sequence tiles

_NC = None
TRACE = False
LAST_RESULT = None
_ABLATE = None  # None = all phases; else subset of {"kv", "q", "attn", "out"}
_CTX_PROBE = None
_SPLIT = set()  # heterogeneous in-group splits are rejected by HW; keep off


def _on(phase):
    return _ABLATE is None or phase in _ABLATE


def _mha(ctx, tc, xqT, xkT, xvT, wqT, wkT, wvT, woT, bq, bk, bv, out, bench_iters=None):
    nc = tc.nc

    cpool = ctx.enter_context(tc.tile_pool(name="const", bufs=1))
    xpool = ctx.enter_context(tc.tile_pool(name="xin", bufs=16))
    epool = ctx.enter_context(tc.tile_pool(name="exp", bufs=6))
    bpool = ctx.enter_context(tc.tile_pool(name="bcsb", bufs=2))
    opool = ctx.enter_context(tc.tile_pool(name="osb", bufs=3))
    sc_ps = ctx.enter_context(tc.tile_pool(name="scps", bufs=4, space="PSUM"))
    ctx_ps = ctx.enter_context(tc.tile_pool(name="ctxps", bufs=4, space="PSUM"))
    mm_ps = sc_ps
    bc_ps = sc_ps

    # ---- persistent SBUF tensors ----
    ones_row = cpool.tile([1, 512], F32R)
    nc.vector.memset(ones_row[:].bitcast(F32), 1.0)
    sel64 = cpool.tile([65, 64], F32R)          # one-hot: row 64 -> all cols
    nc.vector.memset(sel64[:].bitcast(F32), 0.0)
    nc.vector.memset(sel64[64:65, :].bitcast(F32), 1.0)
    rden = cpool.tile([65, 512], F32R)          # row 64 = denom, rows 0..63 zero
    nc.vector.memset(rden[0:64, :].bitcast(F32), 0.0)

    wq_sb = cpool.tile([128, KT_E * DQ], F32R)  # [128, 2048]: wq_sb[p, n*256+m] = WqT[n*128+p, m]
    wk_sb = cpool.tile([128, KT_E * DQ], F32R)
    wv_sb = cpool.tile([128, KT_E * DQ], F32R)
    for sb, src in ((wq_sb, wqT), (wk_sb, wkT), (wv_sb, wvT)):
        nc.sync.dma_start(
            sb[:].rearrange("p (n m) -> p n m", n=KT_E),
            src.rearrange("(n p) m -> p n m", p=128),
        )
    wo_sb = cpool.tile([128, 2 * EMB], F32R)    # wo_sb[p, n*1024+f] = WoT[n*128+p, f]
    nc.sync.dma_start(
        wo_sb[:].rearrange("p (n m) -> p n m", n=2),
        woT.rearrange("(n p) m -> p n m", p=128),
    )
    bq_sb = cpool.tile([1, DQ], F32R)
    bk_sb = cpool.tile([1, DQ], F32R)
    bv_sb = cpool.tile([1, DQ], F32R)
    for sb, src in ((bq_sb, bq), (bk_sb, bk), (bv_sb, bv)):
        nc.sync.dma_start(sb[:], src[:])

    # results of phase 1/2 kept resident
    kT_sb = cpool.tile([128, 2 * S], F32R)      # [dq-block 2][s 2048]
    qT_sb = cpool.tile([128, 2 * S], F32R)
    ctxT_sb = cpool.tile([128, 2 * S], F32R)
    v_sb = cpool.tile([128, NST * (HPC * 65)], F32R)  # per s-tile: 4 heads x (64 V + ones col)
    nc.vector.memset(
        v_sb[:].bitcast(F32).rearrange("p (t h m) -> p t h m", t=NST, h=HPC)[:, :, :, 64:65],
        1.0,
    )

    if _ABLATE:
        # ablation: un-computed persistent tensors need defined contents
        for t in (kT_sb, qT_sb, ctxT_sb, v_sb):
            nc.vector.memset(t[:].bitcast(F32), 0.001)
        nc.vector.memset(rden[64:65, :].bitcast(F32), 1.0)

    def body():
        _body(tc, nc, xqT, xkT, xvT, out, ones_row, sel64, rden, wq_sb, wk_sb,
              wv_sb, wo_sb, bq_sb, bk_sb, bv_sb, kT_sb, qT_sb, ctxT_sb, v_sb,
              xpool, epool, bpool, opool, mm_ps, sc_ps, ctx_ps, bc_ps)

    if bench_iters:
        hints = (
            mybir.EngineType.PE,
            mybir.EngineType.Activation,
            mybir.EngineType.DVE,
            mybir.EngineType.SP,
            mybir.EngineType.Pool,
        )
        with tc.For_i(0, bench_iters, 1, hint_engines=hints):
            body()
    else:
        body()


def _proj_chain(nc, ps, w_sb, xs, dq, b_sb, ones_row):
    """Q/K projection chain into psum ps: kt=0 full K=128 (group start, marks
    every element written), then K=64 halves on alternating row groups (they
    overlap in the PE array), bias last."""
    col = lambda kt: kt * DQ + dq * 128
    nc.tensor.matmul(ps[:], w_sb[:, col(0): col(0) + 128], xs[0][:],
                     start=True, stop=False)
    for kt in range(1, KT_E):
        if "proj" in _SPLIT:
            for b in (0, 64):
                nc.tensor.matmul(
                    ps[:], w_sb[b:b + 64, col(kt): col(kt) + 128], xs[kt][b:b + 64, :],
                    start=False, stop=False,
                )
        else:
            nc.tensor.matmul(
                ps[:], w_sb[:, col(kt): col(kt) + 128], xs[kt][:],
                start=False, stop=False,
            )
    nc.tensor.matmul(
        ps[:], b_sb[0:1, dq * 128: dq * 128 + 128], ones_row[0:1, :],
        start=False, stop=True,
    )


def _body(tc, nc, xqT, xkT, xvT, out, ones_row, sel64, rden, wq_sb, wk_sb,
          wv_sb, wo_sb, bq_sb, bk_sb, bv_sb, kT_sb, qT_sb, ctxT_sb, v_sb,
          xpool, epool, bpool, opool, mm_ps, sc_ps, ctx_ps, bc_ps):
    # ---- phase 1a: K^T = WkT.T @ XkT (+ bk outer ones) ----
    for qc in range(NQC if _on("kv") else 0):
        xk = []
        for kt in range(KT_E):
            t = xpool.tile([128, 512], F32R, tag="xchunk", name=f"xk_{qc}_{kt}")
            nc.sync.dma_start(t[:], xkT[kt, qc])
            xk.append(t)
        for dq in range(2):
            ps = mm_ps.tile([128, 512], F32, tag="sc", name=f"kps_{qc}_{dq}")
            _proj_chain(nc, ps, wk_sb, xk, dq, bk_sb, ones_row)
            nc.vector.tensor_copy(kT_sb[:, dq * S + qc * 512: dq * S + qc * 512 + 512], ps[:])

    # ---- phase 1b: V = XvT.T @ WvT (+ ones outer bv), scattered per head ----
    for qc in range(NQC if _on("kv") else 0):
        xv = []
        for kt in range(KT_E):
            t = xpool.tile([128, 512], F32R, tag="xchunk", name=f"xv_{qc}_{kt}")
            nc.sync.dma_start(t[:], xvT[kt, qc])
            xv.append(t)
        for sti in range(4):
            st = qc * 4 + sti
            ps = mm_ps.tile([128, 256], F32, tag="sc", name=f"vps_{st}")
            nc.tensor.matmul(
                ps[:], xv[0][:, sti * 128: sti * 128 + 128], wv_sb[:, 0:DQ],
                start=True, stop=False,
            )
            for kt in range(1, KT_E):
                if "v" in _SPLIT:
                    for b in (0, 64):
                        nc.tensor.matmul(
                            ps[:], xv[kt][b:b + 64, sti * 128: sti * 128 + 128],
                            wv_sb[b:b + 64, kt * DQ: kt * DQ + DQ],
                            start=False, stop=False,
                        )
                else:
                    nc.tensor.matmul(
                        ps[:], xv[kt][:, sti * 128: sti * 128 + 128],
                        wv_sb[:, kt * DQ: kt * DQ + DQ],
                        start=False, stop=False,
                    )
            nc.tensor.matmul(
                ps[:], ones_row[0:1, 0:128], bv_sb[0:1, :],
                start=False, stop=True,
            )
            dst = v_sb[:, st * (HPC * 65): (st + 1) * (HPC * 65)]
            nc.vector.tensor_copy(
                dst.rearrange("p (h m) -> p h m", h=HPC)[:, :, 0:64],
                ps[:].rearrange("p (h m) -> p h m", h=HPC),
            )

    # ---- phase 2: per query chunk: Q^T slice, attention, out projection ----
    for qc in range(NQC):
        xq = []
        for kt in range(KT_E if _on("q") else 0):
            t = xpool.tile([128, 512], F32R, tag="xchunk", name=f"xq_{qc}_{kt}")
            nc.sync.dma_start(t[:], xqT[kt, qc])
            xq.append(t)
        for dq in range(2 if _on("q") else 0):
            ps = mm_ps.tile([128, 512], F32, tag="sc", name=f"qps_{qc}_{dq}")
            _proj_chain(nc, ps, wq_sb, xq, dq, bq_sb, ones_row)
            nc.vector.tensor_copy(qT_sb[:, dq * S + qc * 512: dq * S + qc * 512 + 512], ps[:])

        # attention, two heads at a time (row-tiled K=64 matmuls can overlap)
        attn_on = _on("attn") or _on("attn_sc") or _on("attn_exp") or _on("attn_ctx")
        do_exp = _on("attn") or _on("attn_exp") or _on("attn_ctx")
        do_ctx = _on("attn") or _on("attn_ctx")
        do_norm = _on("attn")
        for hp in range(HPC // 2 if attn_on else 0):
            # per head: two homogeneous accumulation chains, one per row group
            # (upper 64 k-rows -> bank A at tile row 0, lower -> bank B at row
            # 64); they execute concurrently in the PE array.  Combined at
            # normalize time.
            cps = [
                [
                    ctx_ps.tile([65, 512], F32, tag="ctx", name=f"ctx_{qc}_{hp}_{i}_{half}")
                    for half in range(2)
                ]
                for i in range(2)
            ]

            def ctx_mms(es, kt):
                for hi in range(2):
                    h = hp * 2 + hi
                    rhs = es[hi][:]
                    vcol = kt * (HPC * 65) + h * 65
                    for half, b in enumerate((0, 64)):
                        nc.tensor.matmul(
                            cps[hi][half][:], v_sb[b:b + 64, vcol: vcol + 65],
                            rhs[b:b + 64, :],
                            start=(kt == 0), stop=(kt == NST - 1),
                        )

            prev = None
            for kt in range(NST):
                es = []
                for hi in range(2):
                    base = 64 * hi
                    blk = hp * S
                    sc = sc_ps.tile([128, 512], F32, tag="sc", name=f"sc_{qc}_{hp}_{kt}_{hi}")
                    nc.tensor.matmul(
                        sc[:],
                        kT_sb[base:base + 64, blk + kt * 128: blk + kt * 128 + 128],
                        qT_sb[base:base + 64, blk + qc * 512: blk + qc * 512 + 512],
                        start=True, stop=True,
                    )
                    if do_exp:
                        e = epool.tile([128, 512], F32R, tag="e", name=f"e_{qc}_{hp}_{kt}_{hi}")
                        nc.scalar.activation(e[:], sc[:], EXP, scale=0.125)
                        es.append(e)
                if do_ctx:
                    if prev is not None:
                        ctx_mms(*prev)
                    prev = (es, kt)
            if do_ctx:
                ctx_mms(*prev)
            for hi in range(2 if do_norm else 0):
                # combine the two half-chains, broadcast the raw denominator to
                # 64 partitions via one-hot matmul, then reciprocal + multiply
                # (normalizes ctx on the way out of PSUM)
                tmpa = bpool.tile([65, 512], F32, tag="tmpa", name=f"tmpa_{qc}_{hp}_{hi}")
                nc.vector.tensor_copy(tmpa[:], cps[hi][0][:])
                tmp = bpool.tile([65, 512], F32, tag="tmp", name=f"tmp_{qc}_{hp}_{hi}")
                nc.vector.tensor_add(tmp[:], cps[hi][1][:], tmpa[:])
                nc.vector.tensor_copy(rden[64:65, :], tmp[64:65, :])
                bps = bc_ps.tile([64, 512], F32, tag="sc", name=f"bc_{qc}_{hp}_{hi}")
                nc.tensor.matmul(bps[:], sel64[:], rden[:], start=True, stop=True)
                brec = bpool.tile([64, 512], F32, tag="br", name=f"br_{qc}_{hp}_{hi}")
                nc.vector.reciprocal(brec[:], bps[:])
                nc.vector.tensor_mul(
                    ctxT_sb[64 * hi: 64 * hi + 64, hp * S + qc * 512: hp * S + qc * 512 + 512],
                    tmp[0:64, :],
                    brec[:],
                )

        # out projection for this chunk's 4 query tiles
        for qt4 in range(4 if _on("out") else 0):
            qt = qc * 4 + qt4
            ot = opool.tile([128, EMB], F32, tag="o", name=f"ot_{qt}")
            for fc in range(2):
                ps = mm_ps.tile([128, 512], F32, tag="sc", name=f"ops_{qt}_{fc}")
                nc.tensor.matmul(
                    ps[:],
                    ctxT_sb[:, qt * 128: qt * 128 + 128],
                    wo_sb[:, fc * 512: fc * 512 + 512],
                    start=True, stop=False,
                )
                if "out" in _SPLIT:
                    for b in (0, 64):
                        nc.tensor.matmul(
                            ps[:],
                            ctxT_sb[b:b + 64, S + qt * 128: S + qt * 128 + 128],
                            wo_sb[b:b + 64, EMB + fc * 512: EMB + fc * 512 + 512],
                            start=False, stop=(b == 64),
                        )
                else:
                    nc.tensor.matmul(
                        ps[:],
                        ctxT_sb[:, S + qt * 128: S + qt * 128 + 128],
                        wo_sb[:, EMB + fc * 512: EMB + fc * 512 + 512],
                        start=False, stop=True,
                    )
                nc.vector.tensor_copy(ot[:, fc * 512: fc * 512 + 512], ps[:])
            nc.gpsimd.dma_start(out[qt * 128:(qt + 1) * 128, :], ot[:])


def _build_nc(bench_iters=None):
    from contextlib import ExitStack

    nc = bacc.Bacc("TRN2", target_bir_lowering=False, debug=False, num_devices=NCORES)
    xqT = nc.dram_tensor("xqT", [KT_E, NQC, 128, 512], F32R, kind="ExternalInput").ap()
    xkT = nc.dram_tensor("xkT", [KT_E, NQC, 128, 512], F32R, kind="ExternalInput").ap()
    xvT = nc.dram_tensor("xvT", [KT_E, NQC, 128, 512], F32R, kind="ExternalInput").ap()
    wqT = nc.dram_tensor("wqT", [EMB, DQ], F32R, kind="ExternalInput").ap()
    wkT = nc.dram_tensor("wkT", [EMB, DQ], F32R, kind="ExternalInput").ap()
    wvT = nc.dram_tensor("wvT", [EMB, DQ], F32R, kind="ExternalInput").ap()
    woT = nc.dram_tensor("woT", [DQ, EMB], F32R, kind="ExternalInput").ap()
    bq = nc.dram_tensor("bq", [1, DQ], F32R, kind="ExternalInput").ap()
    bk = nc.dram_tensor("bk", [1, DQ], F32R, kind="ExternalInput").ap()
    bv = nc.dram_tensor("bv", [1, DQ], F32R, kind="ExternalInput").ap()
    out = nc.dram_tensor("out", [S, EMB], F32, kind="ExternalOutput").ap()

    with ExitStack() as ctx:
        tc = ctx.enter_context(tile.TileContext(nc))
        _mha(ctx, tc, xqT, xkT, xvT, wqT, wkT, wvT, woT, bq, bk, bv, out,
             bench_iters=bench_iters)
    nc.compile()
    return nc


def _chunk_major(x):
    """[S, EMB] -> x.T chunked as [KT_E, NQC, 128, 512] (each chunk contiguous)."""
    xt = x.T  # [EMB, S]
    return np.ascontiguousarray(
        xt.reshape(KT_E, 128, NQC, 512).transpose(0, 2, 1, 3)
    )


def kernel(query, key, value, Wq, bq, Wk, bk, Wv, bv, Wo, bo):
    global _NC, LAST_RESULT
    query, key, value, Wq, bq, Wk, bk, Wv, bv, Wo, bo = (
        np.asarray(a, dtype=np.float32)
        for a in (query, key, value, Wq, bq, Wk, bk, Wv, bv, Wo, bo)
    )
    if _NC is None:
        _NC = _build_nc()

    in_maps = []
    for c in range(NCORES):
        b, g = divmod(c, 4)
        rows = slice(g * DQ, (g + 1) * DQ)
        in_maps.append({
            "xqT": _chunk_major(query[b]),
            "xkT": _chunk_major(key[b]),
            "xvT": _chunk_major(value[b]),
            "wqT": np.ascontiguousarray(Wq[rows].T),
            "wkT": np.ascontiguousarray(Wk[rows].T),
            "wvT": np.ascontiguousarray(Wv[rows].T),
            "woT": np.ascontiguousarray(Wo[:, rows].T),
            "bq": np.ascontiguousarray(bq[rows][None, :]),
            "bk": np.ascontiguousarray(bk[rows][None, :]),
            "bv": np.ascontiguousarray(bv[rows][None, :]),
        })

    res = bass_utils.run_bass_kernel_spmd(
        _NC, in_maps, core_ids=list(range(NCORES)), trace=TRACE
    )
    LAST_RESULT = res

    out = np.zeros((B, S, EMB), np.float32)
    for c in range(NCORES):
        out[c // 4] += res.results[c]["out"]
    out += bo[None, None, :]
    return out
